# revision 11
# baseline (speedup 1.0000x reference)
"""AFT-Full on 8 TRN2 cores — raw Bacc build (no TileContext).

v15 (from v14 24.2us): restructured around three HW facts measured in
the v14 trace + hw_specs:
  * every DMA chain costs ~630ns post + ~650-780ns DGE latency +
    transfer + 900ns sem propagation -> minimize serial posts on the
    critical path (xs0+w1 merged into ONE leading DMA per core).
  * the PE HAM clock gate starts cold (1.2GHz) and un-throttles only
    after ~3.4us of sustained activity -> spliced dummy matmuls warm
    the array during the input-DMA dead time.
  * with a warm PE (~4us of real matmul), DVE's 4-op/segment chain
    becomes the ceiling -> ekv moved to the (idle) GpSimd engine.

Layout: segments are CONTIGUOUS column pairs: seg i covers columns
[s, s+2wd); its A-half [s, s+wd) maps to PSUM partitions 0:64 and
B-half [s+wd, s+2wd) to partitions 64:128 via the shifted-window
weight trick.  Segment widths 64/384/448/128 (small first segment to
start compute early, small last segment to shorten the drain-down).

Engine streams (per core):
  SYNC   : xs1/xs2/xs3 slab DMAs (spliced early), 3 seg out-DMAs +
           seg3-b out-DMA, final completion wait
  SCALAR : xw1/xw2 DMAs (spliced early; xw1 = xs0+kqv-weights merged),
           per-seg exp + 2 PSUM->SBUF copies, seg3-a out-DMA
  TENSOR : warmup dummies (spliced early), per-seg 10 matmuls
  VECTOR : per-seg reciprocal, r, o1 (+ seg3 otb copy)
  GPSIMD : per-seg ekv, final semaphore restore

Semaphores: SXW1/SXW2 (scalar-ring input DMAs), SX1/SX2/SX3 (sync-ring
x slabs), SO0 (first out DMA), SP/SA/SV/SG (matmul/ACT/DVE/Pool ops,
+1 each), SO (remaining out DMAs).  CRITICAL RULE: a DMA's
then_inc(sem,16) is sixteen +1s from sixteen queue engines that do NOT
finish in lockstep, so a shared counter can satisfy wait_ge(16) with a
MIX of increments from two DMAs while neither is complete.  Every
DMA-completion wait therefore references either a single-DMA semaphore
or an exact all-DMAs total.  WAR hazards on the static PSUM banks and
SBUF tensors are covered by the thresholds derived in comments below.
"""

import os
import sys

sys.path.insert(0, "/opt/trn_rl_repo")

import numpy as np

from concourse import bacc, mybir
from concourse.bass_utils import run_bass_kernel_spmd


def _register_recip1_mul():
    """Fused out = recip_approx(in0; 1 Newton pass) * in1 as one custom
    DVE op (6 ALU stages <= 8).  Halves the recip+mul DVE time per
    segment; ~9-bit reciprocal is far inside the rel-err budget."""
    import numpy as np
    from concourse import dve_ops as D
    from concourse.dve_spec import AluOp, Bin, Spec, C0, C1, Src0, Src1, lower
    from concourse.dve_spec import _has_src1 as has_src1
    from concourse.dve_uop import DveOpSpec

    if "RECIP1_MUL_ANT" in D._SUB_OPCODE_FOR_NAME:
        return D.CUSTOM_DVE_OPS_BY_NAME["RECIP1_MUL_ANT"]

    _not_x = Bin(AluOp.BITWISE_NOT, Src0, Src0)
    _y0 = _not_x * C0

    def _ref(in0, in1, c0, c1, c2):
        not_x = (~in0.view(np.int32)).view(np.float32)
        y0 = not_x * c0
        y1 = y0 * (c1 - in0 * y0)
        return y1 * in1

    spec = Spec(body=(_y0 * (C1 - Src0 * _y0)) * Src1, reference=_ref)
    op = D.DveOp("RECIP1_MUL_ANT", spec, subdim=False, uops_sha={})
    row = max(D._SUB_OPCODE_FOR_NAME.values()) + 1
    assert row < 0x20
    D._SUB_OPCODE_FOR_NAME[op.name] = row
    for ver in ("v3", "v4"):
        s = DveOpSpec(name=op.name, opcode=row, uops=lower(spec, ver=ver),
                      rd1_en=has_src1(spec))
        op.uops_sha[ver] = s.sha(ver)
    D.OPS.append(op)
    D.CUSTOM_DVE_SPECS[op.name] = spec
    D.CUSTOM_DVE_OPS_BY_NAME = getattr(D, "CUSTOM_DVE_OPS_BY_NAME", {})
    D.CUSTOM_DVE_OPS_BY_NAME[op.name] = op
    return op

BS, C, HH, WW = 4, 128, 64, 64
T = HH * WW
IC = C // 2
NCORES = 8
NCOL = BS * T // NCORES   # 2048
F = 512
WC = 768          # packed weight cols: [Zk K Zk](192) eB(128) Wm(128) [Zq Q Zq V Zq](320)
XW = 128 + WC     # xw tensor: [xs0(128) | w(768)]
C1E = 576         # first scalar-ring chunk: xs0 + K + eB + Wm

_f32 = mybir.dt.float32
_bf16 = mybir.dt.bfloat16

_cached = {}


def _install_ntff_hook():
    import types

    if "antenv.axon_hooks" in sys.modules:
        return
    mod = types.ModuleType("antenv.axon_hooks")
    state = {"hook": None}
    mod.set_axon_ntff_profile_hook = lambda h: state.update(hook=h)
    mod.get_axon_ntff_profile_hook = lambda: state["hook"]
    sys.modules["antenv.axon_hooks"] = mod
    try:
        sys.path.insert(0, "/root/.axon_site")
        from trn_agent_boot.trn_boot import _ntff_profile_via_ctypes

        hook = _ntff_profile_via_ctypes("/opt/axon/libaxon_pjrt.so")
        if hook is not None:
            mod.set_axon_ntff_profile_hook(hook)
    except Exception as e:
        print(f"ntff hook install failed: {e}", file=sys.stderr)


# segment widths; seg i covers x columns [base, base+2*wd)
WIDTHS = [64, 256, 512, 192]
BASES = [0, 128, 640, 1664]
NS = len(WIDTHS)

# dummy-warmup matmul moving widths (cold PE: ~0.85ns/col)
WARMUP = [512, 512, 512, 512, 512, 256]


def _splice_early(nc, early):
    """Move captured instructions to the top of each engine's stream,
    right after that engine's leading barrier Drain, so they run during
    the framework preamble instead of after the all-engine barrier."""
    raw = [bi.ins for bi in early]
    raw_ids = {id(r) for r in raw}
    f = nc.main_func
    for b in f.blocks:
        b.instructions[:] = [i for i in b.instructions if id(i) not in raw_ids]
    entry = f.blocks[0]
    ins_pt = {}
    for idx, ins in enumerate(entry.instructions):
        if isinstance(ins, mybir.InstDrain) and ins.engine not in ins_pt:
            ins_pt[ins.engine] = idx + 1
    for r in raw:
        at = ins_pt.get(r.engine, 0)
        entry.instructions.insert(at, r)
        if r.engine not in ins_pt:
            ins_pt[r.engine] = 0
        for e in ins_pt:
            if ins_pt[e] >= at:
                ins_pt[e] += 1
        ins_pt[r.engine] = at + 1


def _build():
    nc = bacc.Bacc("TRN2", target_bir_lowering=False, debug=False)
    # xw = [x cols 0:128 | packed weights]; xr = x cols 128:2048
    xw_ext = nc.dram_tensor("xw", [C, XW], _bf16, kind="ExternalInput")
    xr_ext = nc.dram_tensor("xr", [C, NCOL - 128], _bf16, kind="ExternalInput")
    out_ext = nc.dram_tensor("out", [C, NCOL], _bf16, kind="ExternalOutput")

    EXP = mybir.ActivationFunctionType.Exp

    # static SBUF tensors
    xw = nc.alloc_sbuf_tensor("xw_sb", [C, XW], _bf16)     # [xs0 | w]
    xs = nc.alloc_sbuf_tensor("xs_sb", [C, NCOL - 128], _bf16)  # x cols 128:
    ek = nc.alloc_sbuf_tensor("ek", [C, F], _bf16)
    ekv = nc.alloc_sbuf_tensor("ekv", [C, F], _bf16)
    rden = nc.alloc_sbuf_tensor("rden", [C, F], _f32)
    rr = nc.alloc_sbuf_tensor("rr", [C, F], _f32)
    o1 = nc.alloc_sbuf_tensor("o1", [C, F], _bf16)
    ot = [
        nc.alloc_sbuf_tensor("ot0", [C, 2 * F], _bf16),
        nc.alloc_sbuf_tensor("ot1", [C, 2 * F], _bf16),
        nc.alloc_sbuf_tensor("ot2", [C, 2 * F], _bf16),
    ]

    # static PSUM banks
    pk = nc.alloc_psum_tensor("pk", [C, F], _f32)
    pq0 = nc.alloc_psum_tensor("pq0", [C, F], _f32)
    pq1 = nc.alloc_psum_tensor("pq1", [C, F], _f32)
    pqs = [pq0, pq1]
    pv = nc.alloc_psum_tensor("pv", [C, F], _f32)
    pden = nc.alloc_psum_tensor("pden", [C, F], _f32)
    pnum = nc.alloc_psum_tensor("pnum", [C, F], _f32)
    poa = nc.alloc_psum_tensor("poa", [C, F], _f32)
    pob = nc.alloc_psum_tensor("pob", [C, F], _f32)

    SC1 = nc.alloc_semaphore("SC1")
    SC2 = nc.alloc_semaphore("SC2")
    SX1 = nc.alloc_semaphore("SX1")
    SX2 = nc.alloc_semaphore("SX2")
    SX3 = nc.alloc_semaphore("SX3")
    SO0 = nc.alloc_semaphore("SO0")
    SP = nc.alloc_semaphore("SP")
    SA = nc.alloc_semaphore("SA")
    SV = nc.alloc_semaphore("SV")
    SO = nc.alloc_semaphore("SO")

    # weight slices inside xw (weights start at col 128):
    # [Zk WkT Zk | eB-blkdiag | WmT x2 | Zq WqT Zq WvT Zq]
    wk = (xw[:, 192:320], xw[:, 128:256])
    w_eB = xw[:, 320:448]
    w_m = xw[:, 448:576]
    wq = (xw[:, 640:768], xw[:, 576:704])
    wv = (xw[:, 768:896], xw[:, 704:832])

    def xab(i):
        wd, s = WIDTHS[i], BASES[i]
        if i == 0:
            return xw[:, 0:wd], xw[:, wd:2 * wd]
        s -= 128
        return xs[:, s:s + wd], xs[:, s + wd:s + 2 * wd]

    # --- software-pipelined PE schedule -------------------------------
    # PE order: kqv_0, den_0, num_0, [kqv_1, fin_0, den_1, num_1], ...,
    # fin_3.  fin_i is deferred into segment i+1's slot so the PE has
    # k/q/v work while segment i's DVE chain (recip, r, o1) completes.
    k_done, v_done, den_pos, num_pos = {}, {}, {}, {}
    fa_pos, fb_pos = {}, {}
    pos = 0
    for i in range(NS):
        pos += 2
        k_done[i] = pos
        pos += 2  # q
        pos += 2
        v_done[i] = pos
        if i > 0:
            fa_pos[i - 1] = pos + 1
            fb_pos[i - 1] = pos + 2
            pos += 2
        den_pos[i] = pos + 1
        num_pos[i] = pos + 2
        pos += 2
    fa_pos[NS - 1] = pos + 1
    fb_pos[NS - 1] = pos + 2
    # SA stream order: exp0, [exp_i, ota_{i-1}, otb_{i-1}] ..., ota3
    exp_done, ota_done, otb_done = {}, {}, {}
    sa = 1
    exp_done[0] = sa
    for i in range(1, NS):
        sa += 1; exp_done[i] = sa
        sa += 1; ota_done[i - 1] = sa
        sa += 1; otb_done[i - 1] = sa
    sa += 1; ota_done[NS - 1] = sa

    early = []  # instructions to splice ahead of the all-engine barrier

    with nc.Block(no_gpsimd_drain=True) as block:

        @block.sync
        def _(sync):
            early.append(
                sync.dma_start(xw[:, C1E:XW], xw_ext[:, C1E:XW]
                               ).then_inc(SC2, 16))
            for lo, hi, sem in [(128, 640, SX1), (1664, 2048, SX3)]:
                early.append(
                    sync.dma_start(xs[:, lo - 128:hi - 128],
                                   xr_ext[:, lo - 128:hi - 128]
                                   ).then_inc(sem, 16))
            for i in range(NS - 1):
                wd, s = WIDTHS[i], BASES[i]
                sync.wait_ge(SA, otb_done[i])
                sync.dma_start(
                    out_ext[:, s:s + 2 * wd], ot[i % 3][:, 0:2 * wd]
                ).then_inc(SO0 if i == 0 else SO, 16)
            # tail segment: b-half here as soon as the DVE copy lands
            # (a-half goes out on the scalar ring)
            i, wd, s = NS - 1, WIDTHS[NS - 1], BASES[NS - 1]
            sync.wait_ge(SV, 3 * NS + 1)  # otb3 copy (on DVE)
            sync.dma_start(
                out_ext[:, s + wd:s + 2 * wd], ot[i % 3][:, wd:2 * wd]
            ).then_inc(SO, 16)
            sync.wait_ge(SO0, 16)
            sync.wait_ge(SO, 64)

        @block.gpsimd
        def _(gpsimd):
            # restore semaphores for potential NEFF re-execution
            gpsimd.wait_ge(SO0, 16)
            gpsimd.wait_ge(SO, 64)
            gpsimd.sem_clear(range(SC1.num, SO.num + 1))

        @block.scalar
        def _(scalar):
            early.append(
                scalar.dma_start(xw[:, 0:C1E], xw_ext[:, 0:C1E]
                                 ).then_inc(SC1, 16))
            early.append(
                scalar.dma_start(xs[:, 640 - 128:1664 - 128],
                                 xr_ext[:, 640 - 128:1664 - 128]
                                 ).then_inc(SX2, 16))

            def exp_op(i):
                wd = WIDTHS[i]
                scalar.wait_ge(SP, k_done[i])
                scalar.activation(ek[:, 0:wd], pk[:, 0:wd], EXP).then_inc(SA)

            def copies(i):
                wd = WIDTHS[i]
                scalar.wait_ge(SP, fa_pos[i])
                scalar.copy(ot[i % 3][:, 0:wd], poa[:, 0:wd]).then_inc(SA)
                scalar.wait_ge(SP, fb_pos[i])
                scalar.copy(ot[i % 3][:, wd:2 * wd], pob[:, 0:wd]).then_inc(SA)

            exp_op(0)
            for i in range(1, NS):
                exp_op(i)
                copies(i - 1)
            # tail segment: only the a-half copy here (b-half on DVE),
            # then its out-DMA on this ring.
            i, wd, s = NS - 1, WIDTHS[NS - 1], BASES[NS - 1]
            scalar.wait_ge(SP, fa_pos[i])
            scalar.wait_ge(SO0, 16)  # WAR: ot[0] read by seg-0 DMA
            scalar.copy(ot[i % 3][:, 0:wd], poa[:, 0:wd]).then_inc(SA)
            scalar.dma_start(
                out_ext[:, s:s + wd], ot[i % 3][:, 0:wd]
            ).then_inc(SO, 16)

        @block.tensor
        def _(tensor):
            # HAM warm-up: junk matmuls on never-DMAed SBUF (ot2) while
            # the input DMAs are in flight; no sem updates, overwritten
            # PSUM (poa) is first really written by fins(0) w/ start=True.
            warm_splice = bool(int(os.environ.get("AFT_WARMUP_SPLICE", "1")))
            for n in WARMUP:
                mm = tensor.matmul(poa[:, 0:n], ot[2][:, 0:128], ot[2][:, 0:n])
                if warm_splice:
                    early.append(mm)

            def kqv(i):
                wd, s = WIDTHS[i], BASES[i]
                xa, xb = xab(i)
                if i == 0:
                    tensor.wait_ge(SC1, 16)
                else:
                    tensor.wait_ge([SX1, SX2, SX3][i - 1], 16)
                    tensor.wait_ge(SA, exp_done[i - 1])  # WAR pk vs exp
                tensor.matmul(pk[:, 0:wd], wk[0], xa, start=True, stop=False
                              ).then_inc(SP)
                tensor.matmul(pk[:, 0:wd], wk[1], xb, start=False, stop=True
                              ).then_inc(SP)
                if i == 0:
                    tensor.wait_ge(SC2, 16)
                if i >= 2:
                    tensor.wait_ge(SV, 3 * (i - 2) + 3)  # WAR pq[i%2] vs o1(i-2)
                pq = pqs[i % 2]
                tensor.matmul(pq[:, 0:wd], wq[0], xa, start=True, stop=False
                              ).then_inc(SP)
                tensor.matmul(pq[:, 0:wd], wq[1], xb, start=False, stop=True
                              ).then_inc(SP)
                if i >= 1:
                    tensor.wait_ge(SV, 3 * (i - 1) + 1)  # WAR pv vs ekv(i-1)
                tensor.matmul(pv[:, 0:wd], wv[0], xa, start=True, stop=False
                              ).then_inc(SP)
                tensor.matmul(pv[:, 0:wd], wv[1], xb, start=False, stop=True
                              ).then_inc(SP)

            def dennum(i):
                wd = WIDTHS[i]
                tensor.wait_ge(SA, exp_done[i])  # ek ready (covers WAR)
                if i >= 1:
                    tensor.wait_ge(SV, 3 * (i - 1) + 2)  # WAR pden/pnum vs rm
                tensor.matmul(pden[:, 0:wd], w_eB, ek[:, 0:wd]).then_inc(SP)
                tensor.wait_ge(SV, 3 * i + 1)  # ekv ready (covers WAR pnum)
                tensor.matmul(pnum[:, 0:wd], w_eB, ekv[:, 0:wd]).then_inc(SP)

            def fins(i):
                wd = WIDTHS[i]
                tensor.wait_ge(SV, 3 * i + 3)  # o1 ready
                if i >= 1:
                    tensor.wait_ge(SA, ota_done[i - 1])  # WAR poa vs ota
                tensor.matmul(poa[:, 0:wd], w_m[0:64, :], o1[0:64, 0:wd]
                              ).then_inc(SP)
                if i >= 1:
                    tensor.wait_ge(SA, otb_done[i - 1])  # WAR pob vs otb
                tensor.matmul(pob[:, 0:wd], w_m[64:128, :], o1[64:128, 0:wd]
                              ).then_inc(SP)

            for i in range(NS):
                kqv(i)
                if i > 0:
                    fins(i - 1)
                dennum(i)
            fins(NS - 1)

        @block.vector
        def _(vector):
            from concourse.dve_ops import RECIP_APPROX_FAST_CONSTS as _RC
            fuse = bool(int(os.environ.get("AFT_FUSE", "0")))
            rm_op = _register_recip1_mul() if fuse else None
            for i in range(NS):
                wd = WIDTHS[i]
                vector.wait_ge(SA, exp_done[i])
                vector.wait_ge(SP, v_done[i])
                vector.tensor_mul(ekv[:, 0:wd], ek[:, 0:wd], pv[:, 0:wd]
                                  ).then_inc(SV)
                vector.wait_ge(SP, num_pos[i])
                if fuse:
                    vector._custom_dve(rm_op, out=rr[:, 0:wd],
                                       in0=pden[:, 0:wd], in1=pnum[:, 0:wd],
                                       s0=_RC["s0"], s1=_RC["s1"]
                                       ).then_inc(SV)
                else:
                    # two ops, ONE SV inc (on the mul) so wait counts match
                    vector.reciprocal_approx_fast(rden[:, 0:wd], pden[:, 0:wd])
                    vector.tensor_mul(rr[:, 0:wd], rden[:, 0:wd],
                                      pnum[:, 0:wd]).then_inc(SV)
                vector.tensor_mul(o1[:, 0:wd], rr[:, 0:wd],
                                  pqs[i % 2][:, 0:wd]).then_inc(SV)
                if i == NS - 1:
                    vector.wait_ge(SP, fb_pos[i])
                    vector.wait_ge(SO0, 16)  # WAR: ot[0] read by seg-0 DMA
                    vector.tensor_copy(ot[i % 3][:, wd:2 * wd], pob[:, 0:wd]
                                       ).then_inc(SV)

    if bool(int(os.environ.get("AFT_SPLICE", "1"))):
        _splice_early(nc, early)

    nc.compile()
    return nc


def _pack_weights(Wq, Wk, Wv, B, Wm):
    eB = np.exp(B)
    w = np.zeros((C, WC), np.float32)
    w[:, 64:128] = Wk.T
    w[0:IC, 192:256] = eB.T
    w[IC:C, 256:320] = eB.T
    w[0:IC, 320:448] = Wm.T
    w[IC:C, 320:448] = Wm.T
    w[:, 512:576] = Wq.T
    w[:, 640:704] = Wv.T
    return w


def kernel(x, Wq, Wk, Wv, B, Wm):
    import ml_dtypes

    x = np.ascontiguousarray(np.asarray(x, dtype=np.float32))
    Wq = np.asarray(Wq, dtype=np.float32)
    Wk = np.asarray(Wk, dtype=np.float32)
    Wv = np.asarray(Wv, dtype=np.float32)
    B = np.asarray(B, dtype=np.float32)
    Wm = np.asarray(Wm, dtype=np.float32)

    xf = x.reshape(BS, C, T)
    per_batch = NCORES // BS
    w = _pack_weights(Wq, Wk, Wv, B, Wm)

    in_maps = []
    for core in range(NCORES):
        b, j = divmod(core, per_batch)
        shard = xf[b, :, j * NCOL:(j + 1) * NCOL]
        xw = np.concatenate([shard[:, 0:128], w], axis=1)
        in_maps.append({
            "xw": np.ascontiguousarray(xw.astype(ml_dtypes.bfloat16)),
            "xr": np.ascontiguousarray(
                shard[:, 128:].astype(ml_dtypes.bfloat16)),
        })

    if "nc" not in _cached:
        _cached["nc"] = _build()
    nc = _cached["nc"]

    trace = bool(int(os.environ.get("AFT_TRACE", "0")))
    if trace:
        _install_ntff_hook()
    try:
        res = run_bass_kernel_spmd(
            nc, in_maps, core_ids=list(range(NCORES)), trace=trace
        )
    except Exception as e:  # rare transient device wedge: retry once
        print(f"run_bass_kernel_spmd failed ({e}); retrying", file=sys.stderr)
        import time

        time.sleep(3.0)
        res = run_bass_kernel_spmd(
            nc, in_maps, core_ids=list(range(NCORES)), trace=trace
        )
    kernel.last_exec_time_ns = res.exec_time_ns
    kernel.last_results = res

    out = np.empty((BS, C, T), np.float32)
    for core in range(NCORES):
        b, j = divmod(core, per_batch)
        out[b, :, j * NCOL:(j + 1) * NCOL] = np.asarray(
            res.results[core]["out"], dtype=np.float32)
    return out.reshape(BS, C, HH, WW)


kernel.last_exec_time_ns = None
kernel.last_results = None


# revision 12
# speedup vs baseline: 1.0048x; 1.0048x over previous
"""AFT-Full on 8 TRN2 cores — raw Bacc build (no TileContext).

v15 (from v14 24.2us): restructured around three HW facts measured in
the v14 trace + hw_specs:
  * every DMA chain costs ~630ns post + ~650-780ns DGE latency +
    transfer + 900ns sem propagation -> minimize serial posts on the
    critical path (xs0+w1 merged into ONE leading DMA per core).
  * the PE HAM clock gate starts cold (1.2GHz) and un-throttles only
    after ~3.4us of sustained activity -> spliced dummy matmuls warm
    the array during the input-DMA dead time.
  * with a warm PE (~4us of real matmul), DVE's 4-op/segment chain
    becomes the ceiling -> ekv moved to the (idle) GpSimd engine.

Layout: segments are CONTIGUOUS column pairs: seg i covers columns
[s, s+2wd); its A-half [s, s+wd) maps to PSUM partitions 0:64 and
B-half [s+wd, s+2wd) to partitions 64:128 via the shifted-window
weight trick.  Segment widths 64/384/448/128 (small first segment to
start compute early, small last segment to shorten the drain-down).

Engine streams (per core):
  SYNC   : xs1/xs2/xs3 slab DMAs (spliced early), 3 seg out-DMAs +
           seg3-b out-DMA, final completion wait
  SCALAR : xw1/xw2 DMAs (spliced early; xw1 = xs0+kqv-weights merged),
           per-seg exp + 2 PSUM->SBUF copies, seg3-a out-DMA
  TENSOR : warmup dummies (spliced early), per-seg 10 matmuls
  VECTOR : per-seg reciprocal, r, o1 (+ seg3 otb copy)
  GPSIMD : per-seg ekv, final semaphore restore

Semaphores: SXW1/SXW2 (scalar-ring input DMAs), SX1/SX2/SX3 (sync-ring
x slabs), SO0 (first out DMA), SP/SA/SV/SG (matmul/ACT/DVE/Pool ops,
+1 each), SO (remaining out DMAs).  CRITICAL RULE: a DMA's
then_inc(sem,16) is sixteen +1s from sixteen queue engines that do NOT
finish in lockstep, so a shared counter can satisfy wait_ge(16) with a
MIX of increments from two DMAs while neither is complete.  Every
DMA-completion wait therefore references either a single-DMA semaphore
or an exact all-DMAs total.  WAR hazards on the static PSUM banks and
SBUF tensors are covered by the thresholds derived in comments below.
"""

import os
import sys

sys.path.insert(0, "/opt/trn_rl_repo")

import numpy as np

from concourse import bacc, mybir
from concourse.bass_utils import run_bass_kernel_spmd


def _register_recip1_mul():
    """Fused out = recip_approx(in0; 1 Newton pass) * in1 as one custom
    DVE op (6 ALU stages <= 8).  Halves the recip+mul DVE time per
    segment; ~9-bit reciprocal is far inside the rel-err budget."""
    import numpy as np
    from concourse import dve_ops as D
    from concourse.dve_spec import AluOp, Bin, Spec, C0, C1, Src0, Src1, lower
    from concourse.dve_spec import _has_src1 as has_src1
    from concourse.dve_uop import DveOpSpec

    if "RECIP1_MUL_ANT" in D._SUB_OPCODE_FOR_NAME:
        return D.CUSTOM_DVE_OPS_BY_NAME["RECIP1_MUL_ANT"]

    _not_x = Bin(AluOp.BITWISE_NOT, Src0, Src0)
    _y0 = _not_x * C0

    def _ref(in0, in1, c0, c1, c2):
        not_x = (~in0.view(np.int32)).view(np.float32)
        y0 = not_x * c0
        y1 = y0 * (c1 - in0 * y0)
        return y1 * in1

    spec = Spec(body=(_y0 * (C1 - Src0 * _y0)) * Src1, reference=_ref)
    op = D.DveOp("RECIP1_MUL_ANT", spec, subdim=False, uops_sha={})
    row = max(D._SUB_OPCODE_FOR_NAME.values()) + 1
    assert row < 0x20
    D._SUB_OPCODE_FOR_NAME[op.name] = row
    for ver in ("v3", "v4"):
        s = DveOpSpec(name=op.name, opcode=row, uops=lower(spec, ver=ver),
                      rd1_en=has_src1(spec))
        op.uops_sha[ver] = s.sha(ver)
    D.OPS.append(op)
    D.CUSTOM_DVE_SPECS[op.name] = spec
    D.CUSTOM_DVE_OPS_BY_NAME = getattr(D, "CUSTOM_DVE_OPS_BY_NAME", {})
    D.CUSTOM_DVE_OPS_BY_NAME[op.name] = op
    return op

BS, C, HH, WW = 4, 128, 64, 64
T = HH * WW
IC = C // 2
NCORES = 8
NCOL = BS * T // NCORES   # 2048
F = 512
WC = 768          # packed weight cols: [Zk K Zk](192) eB(128) Wm(128) [Zq Q Zq V Zq](320)
XW = 128 + WC     # xw tensor: [xs0(128) | w(768)]
C1E = 576         # first scalar-ring chunk: xs0 + K + eB + Wm

_f32 = mybir.dt.float32
_bf16 = mybir.dt.bfloat16

_cached = {}


def _install_ntff_hook():
    import types

    if "antenv.axon_hooks" in sys.modules:
        return
    mod = types.ModuleType("antenv.axon_hooks")
    state = {"hook": None}
    mod.set_axon_ntff_profile_hook = lambda h: state.update(hook=h)
    mod.get_axon_ntff_profile_hook = lambda: state["hook"]
    sys.modules["antenv.axon_hooks"] = mod
    try:
        sys.path.insert(0, "/root/.axon_site")
        from trn_agent_boot.trn_boot import _ntff_profile_via_ctypes

        hook = _ntff_profile_via_ctypes("/opt/axon/libaxon_pjrt.so")
        if hook is not None:
            mod.set_axon_ntff_profile_hook(hook)
    except Exception as e:
        print(f"ntff hook install failed: {e}", file=sys.stderr)


# segment widths; seg i covers x columns [base, base+2*wd)
WIDTHS = [64, 256, 448, 256]
BASES = [0, 128, 640, 1536]
NS = len(WIDTHS)

# dummy-warmup matmul moving widths (cold PE: ~0.85ns/col)
WARMUP = [512, 512, 512, 512, 512, 256]


def _splice_early(nc, early):
    """Move captured instructions to the top of each engine's stream,
    right after that engine's leading barrier Drain, so they run during
    the framework preamble instead of after the all-engine barrier."""
    raw = [bi.ins for bi in early]
    raw_ids = {id(r) for r in raw}
    f = nc.main_func
    for b in f.blocks:
        b.instructions[:] = [i for i in b.instructions if id(i) not in raw_ids]
    entry = f.blocks[0]
    ins_pt = {}
    for idx, ins in enumerate(entry.instructions):
        if isinstance(ins, mybir.InstDrain) and ins.engine not in ins_pt:
            ins_pt[ins.engine] = idx + 1
    for r in raw:
        at = ins_pt.get(r.engine, 0)
        entry.instructions.insert(at, r)
        if r.engine not in ins_pt:
            ins_pt[r.engine] = 0
        for e in ins_pt:
            if ins_pt[e] >= at:
                ins_pt[e] += 1
        ins_pt[r.engine] = at + 1


def _build():
    nc = bacc.Bacc("TRN2", target_bir_lowering=False, debug=False)
    # xw = [x cols 0:128 | packed weights]; xr = x cols 128:2048
    xw_ext = nc.dram_tensor("xw", [C, XW], _bf16, kind="ExternalInput")
    xr_ext = nc.dram_tensor("xr", [C, NCOL - 128], _bf16, kind="ExternalInput")
    out_ext = nc.dram_tensor("out", [C, NCOL], _bf16, kind="ExternalOutput")

    EXP = mybir.ActivationFunctionType.Exp

    # static SBUF tensors
    xw = nc.alloc_sbuf_tensor("xw_sb", [C, XW], _bf16)     # [xs0 | w]
    xs = nc.alloc_sbuf_tensor("xs_sb", [C, NCOL - 128], _bf16)  # x cols 128:
    ek = nc.alloc_sbuf_tensor("ek", [C, F], _bf16)
    ekv = nc.alloc_sbuf_tensor("ekv", [C, F], _bf16)
    rden = nc.alloc_sbuf_tensor("rden", [C, F], _f32)
    rr = nc.alloc_sbuf_tensor("rr", [C, F], _f32)
    o1 = nc.alloc_sbuf_tensor("o1", [C, F], _bf16)
    ot = [
        nc.alloc_sbuf_tensor("ot0", [C, 2 * F], _bf16),
        nc.alloc_sbuf_tensor("ot1", [C, 2 * F], _bf16),
        nc.alloc_sbuf_tensor("ot2", [C, 2 * F], _bf16),
    ]

    # static PSUM banks
    pk = nc.alloc_psum_tensor("pk", [C, F], _f32)
    pq0 = nc.alloc_psum_tensor("pq0", [C, F], _f32)
    pq1 = nc.alloc_psum_tensor("pq1", [C, F], _f32)
    pqs = [pq0, pq1]
    pv = nc.alloc_psum_tensor("pv", [C, F], _f32)
    pden = nc.alloc_psum_tensor("pden", [C, F], _f32)
    pnum = nc.alloc_psum_tensor("pnum", [C, F], _f32)
    poa = nc.alloc_psum_tensor("poa", [C, F], _f32)
    pob = nc.alloc_psum_tensor("pob", [C, F], _f32)

    SC1 = nc.alloc_semaphore("SC1")
    SC2 = nc.alloc_semaphore("SC2")
    SX1 = nc.alloc_semaphore("SX1")
    SX2 = nc.alloc_semaphore("SX2")
    SX3 = nc.alloc_semaphore("SX3")
    SO0 = nc.alloc_semaphore("SO0")
    SP = nc.alloc_semaphore("SP")
    SA = nc.alloc_semaphore("SA")
    SV = nc.alloc_semaphore("SV")
    SO = nc.alloc_semaphore("SO")

    # weight slices inside xw (weights start at col 128):
    # [Zk WkT Zk | eB-blkdiag | WmT x2 | Zq WqT Zq WvT Zq]
    wk = (xw[:, 192:320], xw[:, 128:256])
    w_eB = xw[:, 320:448]
    w_m = xw[:, 448:576]
    wq = (xw[:, 640:768], xw[:, 576:704])
    wv = (xw[:, 768:896], xw[:, 704:832])

    def xab(i):
        wd, s = WIDTHS[i], BASES[i]
        if i == 0:
            return xw[:, 0:wd], xw[:, wd:2 * wd]
        s -= 128
        return xs[:, s:s + wd], xs[:, s + wd:s + 2 * wd]

    # --- software-pipelined PE schedule -------------------------------
    # PE order: kqv_0, den_0, num_0, [kqv_1, fin_0, den_1, num_1], ...,
    # fin_3.  fin_i is deferred into segment i+1's slot so the PE has
    # k/q/v work while segment i's DVE chain (recip, r, o1) completes.
    k_done, v_done, den_pos, num_pos = {}, {}, {}, {}
    fa_pos, fb_pos = {}, {}
    pos = 0
    for i in range(NS):
        pos += 2
        k_done[i] = pos
        pos += 2  # q
        pos += 2
        v_done[i] = pos
        if i > 0:
            fa_pos[i - 1] = pos + 1
            fb_pos[i - 1] = pos + 2
            pos += 2
        den_pos[i] = pos + 1
        num_pos[i] = pos + 2
        pos += 2
    fa_pos[NS - 1] = pos + 1
    fb_pos[NS - 1] = pos + 2
    # SA stream order: exp0, [exp_i, ota_{i-1}, otb_{i-1}] ..., ota3
    exp_done, ota_done, otb_done = {}, {}, {}
    sa = 1
    exp_done[0] = sa
    for i in range(1, NS):
        sa += 1; exp_done[i] = sa
        sa += 1; ota_done[i - 1] = sa
        sa += 1; otb_done[i - 1] = sa
    sa += 1; ota_done[NS - 1] = sa

    early = []  # instructions to splice ahead of the all-engine barrier

    with nc.Block() as block:

        @block.sync
        def _(sync):
            early.append(
                sync.dma_start(xw[:, C1E:XW], xw_ext[:, C1E:XW]
                               ).then_inc(SC2, 16))
            for lo, hi, sem in [(128, 640, SX1), (1536, 2048, SX3)]:
                early.append(
                    sync.dma_start(xs[:, lo - 128:hi - 128],
                                   xr_ext[:, lo - 128:hi - 128]
                                   ).then_inc(sem, 16))
            for i in range(NS - 1):
                wd, s = WIDTHS[i], BASES[i]
                sync.wait_ge(SA, otb_done[i])
                sync.dma_start(
                    out_ext[:, s:s + 2 * wd], ot[i % 3][:, 0:2 * wd]
                ).then_inc(SO0 if i == 0 else SO, 16)
            # tail segment: b-half here as soon as the DVE copy lands
            # (a-half goes out on the scalar ring)
            i, wd, s = NS - 1, WIDTHS[NS - 1], BASES[NS - 1]
            sync.wait_ge(SV, 3 * NS + 1)  # otb3 copy (on DVE)
            sync.dma_start(
                out_ext[:, s + wd:s + 2 * wd], ot[i % 3][:, wd:2 * wd]
            ).then_inc(SO, 16)
            sync.wait_ge(SO0, 16)
            sync.wait_ge(SO, 64)

        @block.gpsimd
        def _(gpsimd):
            # restore semaphores for potential NEFF re-execution
            gpsimd.wait_ge(SO0, 16)
            gpsimd.wait_ge(SO, 64)
            gpsimd.sem_clear(range(SC1.num, SO.num + 1))

        @block.scalar
        def _(scalar):
            early.append(
                scalar.dma_start(xw[:, 0:C1E], xw_ext[:, 0:C1E]
                                 ).then_inc(SC1, 16))
            early.append(
                scalar.dma_start(xs[:, 640 - 128:1536 - 128],
                                 xr_ext[:, 640 - 128:1536 - 128]
                                 ).then_inc(SX2, 16))

            def exp_op(i):
                wd = WIDTHS[i]
                scalar.wait_ge(SP, k_done[i])
                scalar.activation(ek[:, 0:wd], pk[:, 0:wd], EXP).then_inc(SA)

            def copies(i):
                wd = WIDTHS[i]
                scalar.wait_ge(SP, fa_pos[i])
                scalar.copy(ot[i % 3][:, 0:wd], poa[:, 0:wd]).then_inc(SA)
                scalar.wait_ge(SP, fb_pos[i])
                scalar.copy(ot[i % 3][:, wd:2 * wd], pob[:, 0:wd]).then_inc(SA)

            exp_op(0)
            for i in range(1, NS):
                exp_op(i)
                copies(i - 1)
            # tail segment: only the a-half copy here (b-half on DVE),
            # then its out-DMA on this ring.
            i, wd, s = NS - 1, WIDTHS[NS - 1], BASES[NS - 1]
            scalar.wait_ge(SP, fa_pos[i])
            scalar.wait_ge(SO0, 16)  # WAR: ot[0] read by seg-0 DMA
            scalar.copy(ot[i % 3][:, 0:wd], poa[:, 0:wd]).then_inc(SA)
            scalar.dma_start(
                out_ext[:, s:s + wd], ot[i % 3][:, 0:wd]
            ).then_inc(SO, 16)

        @block.tensor
        def _(tensor):
            # HAM warm-up: junk matmuls on never-DMAed SBUF (ot2) while
            # the input DMAs are in flight; no sem updates, overwritten
            # PSUM (poa) is first really written by fins(0) w/ start=True.
            warm_splice = bool(int(os.environ.get("AFT_WARMUP_SPLICE", "1")))
            for n in WARMUP:
                mm = tensor.matmul(poa[:, 0:n], ot[2][:, 0:128], ot[2][:, 0:n])
                if warm_splice:
                    early.append(mm)

            def kqv(i):
                wd, s = WIDTHS[i], BASES[i]
                xa, xb = xab(i)
                if i == 0:
                    tensor.wait_ge(SC1, 16)
                else:
                    tensor.wait_ge([SX1, SX2, SX3][i - 1], 16)
                    tensor.wait_ge(SA, exp_done[i - 1])  # WAR pk vs exp
                tensor.matmul(pk[:, 0:wd], wk[0], xa, start=True, stop=False
                              ).then_inc(SP)
                tensor.matmul(pk[:, 0:wd], wk[1], xb, start=False, stop=True
                              ).then_inc(SP)
                if i == 0:
                    tensor.wait_ge(SC2, 16)
                if i >= 2:
                    tensor.wait_ge(SV, 3 * (i - 2) + 3)  # WAR pq[i%2] vs o1(i-2)
                pq = pqs[i % 2]
                tensor.matmul(pq[:, 0:wd], wq[0], xa, start=True, stop=False
                              ).then_inc(SP)
                tensor.matmul(pq[:, 0:wd], wq[1], xb, start=False, stop=True
                              ).then_inc(SP)
                if i >= 1:
                    tensor.wait_ge(SV, 3 * (i - 1) + 1)  # WAR pv vs ekv(i-1)
                tensor.matmul(pv[:, 0:wd], wv[0], xa, start=True, stop=False
                              ).then_inc(SP)
                tensor.matmul(pv[:, 0:wd], wv[1], xb, start=False, stop=True
                              ).then_inc(SP)

            def dennum(i):
                wd = WIDTHS[i]
                tensor.wait_ge(SA, exp_done[i])  # ek ready (covers WAR)
                if i >= 1:
                    tensor.wait_ge(SV, 3 * (i - 1) + 2)  # WAR pden/pnum vs rm
                tensor.matmul(pden[:, 0:wd], w_eB, ek[:, 0:wd]).then_inc(SP)
                tensor.wait_ge(SV, 3 * i + 1)  # ekv ready (covers WAR pnum)
                tensor.matmul(pnum[:, 0:wd], w_eB, ekv[:, 0:wd]).then_inc(SP)

            def fins(i):
                wd = WIDTHS[i]
                tensor.wait_ge(SV, 3 * i + 3)  # o1 ready
                if i >= 1:
                    tensor.wait_ge(SA, ota_done[i - 1])  # WAR poa vs ota
                tensor.matmul(poa[:, 0:wd], w_m[0:64, :], o1[0:64, 0:wd]
                              ).then_inc(SP)
                if i >= 1:
                    tensor.wait_ge(SA, otb_done[i - 1])  # WAR pob vs otb
                tensor.matmul(pob[:, 0:wd], w_m[64:128, :], o1[64:128, 0:wd]
                              ).then_inc(SP)

            for i in range(NS):
                kqv(i)
                if i > 0:
                    fins(i - 1)
                dennum(i)
            fins(NS - 1)

        @block.vector
        def _(vector):
            from concourse.dve_ops import RECIP_APPROX_FAST_CONSTS as _RC
            fuse = bool(int(os.environ.get("AFT_FUSE", "0")))
            rm_op = _register_recip1_mul() if fuse else None
            for i in range(NS):
                wd = WIDTHS[i]
                vector.wait_ge(SA, exp_done[i])
                vector.wait_ge(SP, v_done[i])
                vector.tensor_mul(ekv[:, 0:wd], ek[:, 0:wd], pv[:, 0:wd]
                                  ).then_inc(SV)
                vector.wait_ge(SP, num_pos[i])
                if fuse:
                    vector._custom_dve(rm_op, out=rr[:, 0:wd],
                                       in0=pden[:, 0:wd], in1=pnum[:, 0:wd],
                                       s0=_RC["s0"], s1=_RC["s1"]
                                       ).then_inc(SV)
                else:
                    # two ops, ONE SV inc (on the mul) so wait counts match
                    vector.reciprocal_approx_fast(rden[:, 0:wd], pden[:, 0:wd])
                    vector.tensor_mul(rr[:, 0:wd], rden[:, 0:wd],
                                      pnum[:, 0:wd]).then_inc(SV)
                vector.tensor_mul(o1[:, 0:wd], rr[:, 0:wd],
                                  pqs[i % 2][:, 0:wd]).then_inc(SV)
                if i == NS - 1:
                    vector.wait_ge(SP, fb_pos[i])
                    vector.wait_ge(SO0, 16)  # WAR: ot[0] read by seg-0 DMA
                    vector.tensor_copy(ot[i % 3][:, wd:2 * wd], pob[:, 0:wd]
                                       ).then_inc(SV)

    if bool(int(os.environ.get("AFT_SPLICE", "1"))):
        _splice_early(nc, early)

    nc.compile()
    return nc


def _pack_weights(Wq, Wk, Wv, B, Wm):
    eB = np.exp(B)
    w = np.zeros((C, WC), np.float32)
    w[:, 64:128] = Wk.T
    w[0:IC, 192:256] = eB.T
    w[IC:C, 256:320] = eB.T
    w[0:IC, 320:448] = Wm.T
    w[IC:C, 320:448] = Wm.T
    w[:, 512:576] = Wq.T
    w[:, 640:704] = Wv.T
    return w


def kernel(x, Wq, Wk, Wv, B, Wm):
    import ml_dtypes

    x = np.ascontiguousarray(np.asarray(x, dtype=np.float32))
    Wq = np.asarray(Wq, dtype=np.float32)
    Wk = np.asarray(Wk, dtype=np.float32)
    Wv = np.asarray(Wv, dtype=np.float32)
    B = np.asarray(B, dtype=np.float32)
    Wm = np.asarray(Wm, dtype=np.float32)

    xf = x.reshape(BS, C, T)
    per_batch = NCORES // BS
    w = _pack_weights(Wq, Wk, Wv, B, Wm)

    in_maps = []
    for core in range(NCORES):
        b, j = divmod(core, per_batch)
        shard = xf[b, :, j * NCOL:(j + 1) * NCOL]
        xw = np.concatenate([shard[:, 0:128], w], axis=1)
        in_maps.append({
            "xw": np.ascontiguousarray(xw.astype(ml_dtypes.bfloat16)),
            "xr": np.ascontiguousarray(
                shard[:, 128:].astype(ml_dtypes.bfloat16)),
        })

    if "nc" not in _cached:
        _cached["nc"] = _build()
    nc = _cached["nc"]

    trace = bool(int(os.environ.get("AFT_TRACE", "0")))
    if trace:
        _install_ntff_hook()
    try:
        res = run_bass_kernel_spmd(
            nc, in_maps, core_ids=list(range(NCORES)), trace=trace
        )
    except Exception as e:  # rare transient device wedge: retry once
        print(f"run_bass_kernel_spmd failed ({e}); retrying", file=sys.stderr)
        import time

        time.sleep(3.0)
        res = run_bass_kernel_spmd(
            nc, in_maps, core_ids=list(range(NCORES)), trace=trace
        )
    kernel.last_exec_time_ns = res.exec_time_ns
    kernel.last_results = res

    out = np.empty((BS, C, T), np.float32)
    for core in range(NCORES):
        b, j = divmod(core, per_batch)
        out[b, :, j * NCOL:(j + 1) * NCOL] = np.asarray(
            res.results[core]["out"], dtype=np.float32)
    return out.reshape(BS, C, HH, WW)


kernel.last_exec_time_ns = None
kernel.last_results = None
